# revision 4
# baseline (speedup 1.0000x reference)
"""Trainium2 Bass kernel for nn_BoundaryUnit (gnn_message_passing).

Computation (per batch b):
    q  = f_b @ Wq.T + bq                  [N,D]
    k  = f_w @ Wk.T + bk                  [L,D]
    aw = softmax(scale * q k^T)           [N,L]   (query_mask == ones)
    f_baq = aw @ f_w                      [N,D]
    f_bq  = f_b * (f_baq + f_s)           [N,D]
    A  = softmax(scale * f_bq f_bq^T)     [N,N]   (length_mask == ones)
    f_bb = A @ f_b                        [N,D]
    f_bm = einsum('nm,nmd->nd', A, f_m * sigmoid(f_m * f_s))
    out  = f_bb + f_b + f_bm

Sharding: data-parallel over batch B=8 across the 8 NeuronCores; each core
computes one batch independently (no collectives).

The dominant cost is streaming f_m (16.75 MB/core).  Host pre-transposes
f_m to [m, n, d] so each 2 MB block DMA is made of contiguous 16 KB
per-partition runs (full HBM bandwidth).  The gated product
f_m * sigmoid(f_m * f_s) is computed as silu(f_s * f_m) / f_s — one DVE
multiply + one ACT pass over the stream — with the 1/f_s correction
applied to the small [N,D] accumulator at the end.  The A-weighted
reduction over m runs on the PE as per-row matvecs (stationary = column
of A^T, moving = the silu'd [m, d] slab), accumulating rows directly in
PSUM.
"""

import math
import sys

import numpy as np

sys.path.insert(0, "/opt/trn_rl_repo")

import concourse.bass as bass  # noqa: E402
import concourse.tile as tile  # noqa: E402
from concourse import bass_utils, mybir  # noqa: E402

B, N, L, D = 8, 128, 30, 256
NB = 16            # n-rows of f_m handled per streamed block
NBLK = N // NB     # 8 blocks
SCALE = 1.0 / math.sqrt(D)
F32 = mybir.dt.float32
AF = mybir.ActivationFunctionType
AX = mybir.AxisListType

_CACHED_NC = None


def _legalize_waits(nc):
    """Split multi-wait instructions: this walrus build accepts at most ONE
    sync-wait per data instruction, so move extra waits onto standalone
    InstEventSemaphore (the same lowering wait_ge uses) just before it."""
    for blk in nc.main_func.blocks:
        insts = list(blk.instructions)
        out_list = []
        changed = False
        for inst in insts:
            si = inst.sync_info
            if si is not None and len(si.on_wait) > 1:
                for w in si.on_wait[:-1]:
                    ev = mybir.InstEventSemaphore(
                        name=nc.get_next_instruction_name(), ins=[], outs=[]
                    )
                    ev.engine = inst.engine
                    ev.sync_info = mybir.SyncInfo(on_wait=[w], on_update=[])
                    nc.register_instruction(ev)
                    out_list.append(ev)
                inst.sync_info = mybir.SyncInfo(
                    on_wait=[si.on_wait[-1]], on_update=si.on_update
                )
                changed = True
            out_list.append(inst)
        if changed:
            del blk.instructions[:]
            blk.instructions.extend(out_list)
    return nc


def build_program():
    nc = bass.Bass()

    # Per-core inputs.
    fb = nc.dram_tensor("fb", [N, D], F32, kind="ExternalInput")
    fw = nc.dram_tensor("fw", [L, D], F32, kind="ExternalInput")
    fm = nc.dram_tensor("fm", [N, N, D], F32, kind="ExternalInput")  # [m, n, d]
    fsb = nc.dram_tensor("fsb", [N, D], F32, kind="ExternalInput")   # f_s bcast
    fsi = nc.dram_tensor("fsi", [N, D], F32, kind="ExternalInput")   # 1/f_s bcast
    # Replicated weights. wqT/bqv are pre-scaled by SCALE on the host.
    wqT = nc.dram_tensor("wqT", [D, D], F32, kind="ExternalInput")   # SCALE*Wq.T
    wkT = nc.dram_tensor("wkT", [D, D], F32, kind="ExternalInput")   # Wk.T
    bqv = nc.dram_tensor("bqv", [D, 1], F32, kind="ExternalInput")   # SCALE*bq
    bkv = nc.dram_tensor("bkv", [D, 1], F32, kind="ExternalInput")
    ident = nc.dram_tensor("ident", [128, 128], F32, kind="ExternalInput")

    out = nc.dram_tensor("out", [N, D], F32, kind="ExternalOutput")

    with tile.TileContext(nc) as tc:
        _emit(nc, tc, fb, fw, fm, fsb, fsi, wqT, wkT, bqv, bkv, ident, out)
    return _legalize_waits(nc)


def _emit(nc, tc, fb, fw, fm, fsb, fsi, wqT, wkT, bqv, bkv, ident, out):
    from contextlib import ExitStack

    ctx = ExitStack()
    with ctx:
        consts = ctx.enter_context(tc.tile_pool(name="consts", bufs=1))
        work = ctx.enter_context(tc.tile_pool(name="work", bufs=2))
        fmpool = ctx.enter_context(tc.tile_pool(name="fmblk", bufs=3))
        pp = ctx.enter_context(tc.tile_pool(name="ppsum", bufs=2, space="PSUM"))
        pacc = ctx.enter_context(tc.tile_pool(name="pacc", bufs=1, space="PSUM"))

        # ---- constant loads -------------------------------------------------
        s_fb = consts.tile([N, D], F32, tag="fb")
        nc.sync.dma_start(out=s_fb, in_=fb[:, :])
        s_fw = consts.tile([L, D], F32, tag="fw")
        nc.sync.dma_start(out=s_fw, in_=fw[:, :])
        s_fsb = consts.tile([N, D], F32, tag="fsb")
        nc.sync.dma_start(out=s_fsb, in_=fsb[:, :])
        s_fsi = consts.tile([N, D], F32, tag="fsi")
        nc.sync.dma_start(out=s_fsi, in_=fsi[:, :])
        s_id = consts.tile([128, 128], F32, tag="ident")
        nc.sync.dma_start(out=s_id, in_=ident[:, :])

        s_wq = []
        s_wk = []
        s_bq = []
        s_bk = []
        for c in range(2):
            t = consts.tile([128, D], F32, tag=f"wq{c}")
            nc.sync.dma_start(out=t, in_=wqT[c * 128:(c + 1) * 128, :])
            s_wq.append(t)
            t = consts.tile([128, D], F32, tag=f"wk{c}")
            nc.sync.dma_start(out=t, in_=wkT[c * 128:(c + 1) * 128, :])
            s_wk.append(t)
            t = consts.tile([128, 1], F32, tag=f"bq{c}")
            nc.gpsimd.dma_start(out=t, in_=bqv[c * 128:(c + 1) * 128, :])
            s_bq.append(t)
            t = consts.tile([128, 1], F32, tag=f"bk{c}")
            nc.gpsimd.dma_start(out=t, in_=bkv[c * 128:(c + 1) * 128, :])
            s_bk.append(t)

        # f_s replicated NB times along free dim: [128, NB, D], built from the
        # [N, D] DRAM copy with a stride-0 middle dim (SWDGE broadcast).
        s_fsrep = consts.tile([N, NB, D], F32, tag="fsrep")
        fsb_ap = fsb[:, :]
        fsb_bcast = bass.AP(
            tensor=fsb_ap.tensor,
            offset=fsb_ap.offset,
            ap=[fsb_ap.ap[0], [0, NB], fsb_ap.ap[1]],
        )
        nc.gpsimd.dma_start(out=s_fsrep, in_=fsb_bcast)

        # ---- attention of f_b over f_w -------------------------------------
        # fbT chunks: [d_chunk=128, n=128]
        s_fbT = []
        for c in range(2):
            pt = pp.tile([128, 128], F32, tag="ptrans")
            nc.tensor.transpose(out=pt, in_=s_fb[:, c * 128:(c + 1) * 128],
                                identity=s_id)
            st = work.tile([128, 128], F32, tag=f"fbT{c}")
            nc.vector.tensor_copy(out=st, in_=pt)
            s_fbT.append(st)

        # qT chunks [d'=128, n=128]; q is pre-scaled by SCALE via wqT/bqv.
        s_qT = []
        for mc in range(2):
            pq = pp.tile([128, 128], F32, tag="pmm")
            for kc in range(2):
                nc.tensor.matmul(
                    out=pq,
                    lhsT=s_wq[kc][:, mc * 128:(mc + 1) * 128],
                    rhs=s_fbT[kc],
                    start=(kc == 0),
                    stop=(kc == 1),
                )
            st = work.tile([128, 128], F32, tag=f"qT{mc}")
            nc.scalar.activation(out=st, in_=pq, func=AF.Identity,
                                 bias=s_bq[mc], scale=1.0)
            s_qT.append(st)

        # fwT chunks: [d_chunk=128, l=30]
        s_fwT = []
        for c in range(2):
            pt = pp.tile([128, L], F32, tag="ptrans")
            nc.tensor.transpose(out=pt, in_=s_fw[:, c * 128:(c + 1) * 128],
                                identity=s_id[:L, :L])
            st = work.tile([128, L], F32, tag=f"fwT{c}")
            nc.vector.tensor_copy(out=st, in_=pt)
            s_fwT.append(st)

        # kT chunks [d'=128, l=30]
        s_kT = []
        for mc in range(2):
            pk = pp.tile([128, L], F32, tag="pmm")
            for kc in range(2):
                nc.tensor.matmul(
                    out=pk,
                    lhsT=s_wk[kc][:, mc * 128:(mc + 1) * 128],
                    rhs=s_fwT[kc],
                    start=(kc == 0),
                    stop=(kc == 1),
                )
            st = work.tile([128, L], F32, tag=f"kT{mc}")
            nc.scalar.activation(out=st, in_=pk, func=AF.Identity,
                                 bias=s_bk[mc], scale=1.0)
            s_kT.append(st)

        # aw logits [n=128, l=30] (already scaled by SCALE)
        p_aw = pp.tile([N, L], F32, tag="pmm")
        for kc in range(2):
            nc.tensor.matmul(out=p_aw, lhsT=s_qT[kc], rhs=s_kT[kc],
                             start=(kc == 0), stop=(kc == 1))

        # softmax over l
        mx1 = work.tile([N, 1], F32, tag="mx1")
        nc.vector.reduce_max(out=mx1, in_=p_aw, axis=AX.X)
        nmx1 = work.tile([N, 1], F32, tag="nmx1")
        nc.scalar.mul(nmx1, mx1, -1.0)
        e_aw = work.tile([N, L], F32, tag="eaw")
        nc.scalar.activation(out=e_aw, in_=p_aw, func=AF.Exp,
                             bias=nmx1, scale=1.0)
        sm1 = work.tile([N, 1], F32, tag="sm1")
        nc.vector.reduce_sum(out=sm1, in_=e_aw, axis=AX.X)
        r1 = work.tile([N, 1], F32, tag="r1")
        nc.vector.reciprocal(out=r1, in_=sm1)
        naw = work.tile([N, L], F32, tag="naw")
        nc.vector.tensor_scalar_mul(naw, e_aw, r1)

        # f_baq = naw @ f_w : transpose naw -> [l, n], then PE
        p_awT = pp.tile([L, N], F32, tag="ptrans")
        nc.tensor.transpose(out=p_awT, in_=naw, identity=s_id)
        s_awT = work.tile([L, N], F32, tag="awT")
        nc.vector.tensor_copy(out=s_awT, in_=p_awT)

        p_fbaq = pp.tile([N, D], F32, tag="pmm")
        nc.tensor.matmul(out=p_fbaq, lhsT=s_awT, rhs=s_fw,
                         start=True, stop=True)

        # f_bq = f_b * (f_baq + f_s)
        s_t = work.tile([N, D], F32, tag="t")
        nc.vector.tensor_add(s_t, p_fbaq, s_fsb)
        s_fbq = work.tile([N, D], F32, tag="fbq")
        nc.vector.tensor_mul(s_fbq, s_t, s_fb)

        # A = softmax(SCALE * f_bq f_bq^T) over m
        s_fbqT = []
        for c in range(2):
            pt = pp.tile([128, 128], F32, tag="ptrans")
            nc.tensor.transpose(out=pt, in_=s_fbq[:, c * 128:(c + 1) * 128],
                                identity=s_id)
            st = work.tile([128, 128], F32, tag=f"fbqT{c}")
            nc.vector.tensor_copy(out=st, in_=pt)
            s_fbqT.append(st)

        p_A = pp.tile([N, N], F32, tag="pmm")
        for kc in range(2):
            nc.tensor.matmul(out=p_A, lhsT=s_fbqT[kc], rhs=s_fbqT[kc],
                             start=(kc == 0), stop=(kc == 1))

        mx2 = work.tile([N, 1], F32, tag="mx2")
        nc.vector.reduce_max(out=mx2, in_=p_A, axis=AX.X)
        nmx2 = work.tile([N, 1], F32, tag="nmx2")
        nc.scalar.mul(nmx2, mx2, -SCALE)
        e_A = work.tile([N, N], F32, tag="eA")
        nc.scalar.activation(out=e_A, in_=p_A, func=AF.Exp,
                             bias=nmx2, scale=SCALE)
        sm2 = work.tile([N, 1], F32, tag="sm2")
        nc.vector.reduce_sum(out=sm2, in_=e_A, axis=AX.X)
        r2 = work.tile([N, 1], F32, tag="r2")
        nc.vector.reciprocal(out=r2, in_=sm2)
        s_A = work.tile([N, N], F32, tag="A")
        nc.vector.tensor_scalar_mul(s_A, e_A, r2)

        # A^T for the matvec stationaries and f_bb
        p_AT = pp.tile([N, N], F32, tag="ptrans")
        nc.tensor.transpose(out=p_AT, in_=s_A, identity=s_id)
        s_AT = work.tile([N, N], F32, tag="AT")
        nc.vector.tensor_copy(out=s_AT, in_=p_AT)

        # f_bb = A @ f_b
        p_fbb = pacc.tile([N, D], F32, tag="fbb")
        nc.tensor.matmul(out=p_fbb, lhsT=s_AT, rhs=s_fb, start=True, stop=True)

        # ---- streamed gated aggregation over f_m ---------------------------
        # PE matmul outputs must start at partition 0/32/64, so accumulate
        # f_bm transposed: each matvec writes one column of f_bm^T
        # (stationary = silu'd [m, d_half] slab, moving = A^T column).
        p_fbmT = pacc.tile([128, 2, N], F32, tag="fbmT")
        for j in range(NBLK):
            xt = fmpool.tile([128, NB, D], F32, tag="xt")
            nc.sync.dma_start(out=xt, in_=fm[:, j * NB:(j + 1) * NB, :])
            x2 = xt.rearrange("m n d -> m (n d)")
            nc.vector.tensor_mul(x2, x2, s_fsrep.rearrange("m n d -> m (n d)"))
            nc.scalar.activation(out=x2, in_=x2, func=AF.Silu)
            for i in range(NB):
                n = j * NB + i
                for c in range(2):
                    nc.tensor.matmul(
                        out=p_fbmT[:, c, n:n + 1],
                        lhsT=xt[:, i, c * 128:(c + 1) * 128],
                        rhs=s_AT[:, n:n + 1],
                        start=True,
                        stop=True,
                    )

        # transpose f_bm^T halves back to [n, d]
        s_fbmT = work.tile([128, 2, N], F32, tag="fbmT_s")
        nc.vector.tensor_copy(out=s_fbmT.rearrange("p c n -> p (c n)"),
                              in_=p_fbmT.rearrange("p c n -> p (c n)"))
        p_fbm = pacc.tile([N, D], F32, tag="fbm")
        for c in range(2):
            nc.tensor.transpose(out=p_fbm[:, c * 128:(c + 1) * 128],
                                in_=s_fbmT[:, c, :], identity=s_id)

        # ---- combine: out = f_bb + f_b + f_bm / f_s ------------------------
        s_o1 = work.tile([N, D], F32, tag="o1")
        nc.vector.tensor_mul(s_o1, p_fbm, s_fsi)
        s_o2 = work.tile([N, D], F32, tag="o2")
        nc.vector.tensor_add(s_o2, p_fbb, s_fb)
        s_out = work.tile([N, D], F32, tag="oo")
        nc.vector.tensor_add(s_out, s_o1, s_o2)
        nc.sync.dma_start(out=out[:, :], in_=s_out)


def get_program():
    global _CACHED_NC
    if _CACHED_NC is None:
        _CACHED_NC = build_program()
    return _CACHED_NC


def make_in_maps(inputs):
    f_b = np.ascontiguousarray(np.asarray(inputs["f_b"], np.float32))
    f_w = np.ascontiguousarray(np.asarray(inputs["f_w"], np.float32))
    f_s = np.ascontiguousarray(np.asarray(inputs["f_s"], np.float32))
    f_m = np.asarray(inputs["f_m"], np.float32)
    Wq = np.asarray(inputs["Wq"], np.float32)
    bq = np.asarray(inputs["bq"], np.float32)
    Wk = np.asarray(inputs["Wk"], np.float32)
    bk = np.asarray(inputs["bk"], np.float32)

    wqT = np.ascontiguousarray(Wq.T * SCALE)
    wkT = np.ascontiguousarray(Wk.T)
    bqv = np.ascontiguousarray((bq * SCALE).reshape(D, 1))
    bkv = np.ascontiguousarray(bk.reshape(D, 1))
    ident = np.eye(128, dtype=np.float32)

    in_maps = []
    for b in range(B):
        in_maps.append({
            "fb": f_b[b],
            "fw": f_w[b],
            # [n, m, d] -> [m, n, d] so block DMAs are contiguous 16KB runs
            "fm": np.ascontiguousarray(f_m[b].transpose(1, 0, 2)),
            "fsb": np.ascontiguousarray(np.broadcast_to(f_s[b], (N, D))),
            "fsi": np.ascontiguousarray(np.broadcast_to(1.0 / f_s[b], (N, D))),
            "wqT": wqT,
            "wkT": wkT,
            "bqv": bqv,
            "bkv": bkv,
            "ident": ident,
        })
    return in_maps


def kernel(**inputs) -> np.ndarray:
    nc = get_program()
    in_maps = make_in_maps(inputs)
    res = bass_utils.run_bass_kernel_spmd(nc, in_maps, list(range(B))).results
    return np.stack([np.asarray(res[b]["out"], np.float32) for b in range(B)],
                    axis=0)


# revision 6
# speedup vs baseline: 1.3500x; 1.3500x over previous
"""Trainium2 Bass kernel for nn_BoundaryUnit (gnn_message_passing).

Computation (per batch b):
    q  = f_b @ Wq.T + bq                  [N,D]
    k  = f_w @ Wk.T + bk                  [L,D]
    aw = softmax(scale * q k^T)           [N,L]   (query_mask == ones)
    f_baq = aw @ f_w                      [N,D]
    f_bq  = f_b * (f_baq + f_s)           [N,D]
    A  = softmax(scale * f_bq f_bq^T)     [N,N]   (length_mask == ones)
    f_bb = A @ f_b                        [N,D]
    f_bm = einsum('nm,nmd->nd', A, f_m * sigmoid(f_m * f_s))
    out  = f_bb + f_b + f_bm

Sharding: data-parallel over batch B=8 across the 8 NeuronCores; each core
computes one batch independently (no collectives).

The dominant cost is streaming f_m (16.75 MB/core).  Host pre-transposes
f_m to [m, n, d] so each 2 MB block DMA is made of contiguous 16 KB
per-partition runs (full HBM bandwidth).  The gated product
f_m * sigmoid(f_m * f_s) is computed as silu(f_s * f_m) / f_s — one DVE
multiply + one ACT pass over the stream — with the 1/f_s correction
applied to the small [N,D] accumulator at the end.  The A-weighted
reduction over m runs on the PE as per-row matvecs (stationary = column
of A^T, moving = the silu'd [m, d] slab), accumulating rows directly in
PSUM.
"""

import math
import sys

import numpy as np

sys.path.insert(0, "/opt/trn_rl_repo")

import concourse.bass as bass  # noqa: E402
import concourse.tile as tile  # noqa: E402
from concourse import bass_utils, mybir  # noqa: E402

B, N, L, D = 8, 128, 30, 256
NB = 16            # n-rows of f_m handled per streamed block
NBLK = N // NB     # 8 blocks
SCALE = 1.0 / math.sqrt(D)
F32 = mybir.dt.float32
AF = mybir.ActivationFunctionType
AX = mybir.AxisListType

_CACHED_NC = None


def _legalize_waits(nc):
    """Split multi-wait instructions: this walrus build accepts at most ONE
    sync-wait per data instruction, so move extra waits onto standalone
    InstEventSemaphore (the same lowering wait_ge uses) just before it."""
    for blk in nc.main_func.blocks:
        insts = list(blk.instructions)
        out_list = []
        changed = False
        for inst in insts:
            si = inst.sync_info
            if si is not None and len(si.on_wait) > 1:
                for w in si.on_wait[:-1]:
                    ev = mybir.InstEventSemaphore(
                        name=nc.get_next_instruction_name(), ins=[], outs=[]
                    )
                    ev.engine = inst.engine
                    ev.sync_info = mybir.SyncInfo(on_wait=[w], on_update=[])
                    nc.register_instruction(ev)
                    out_list.append(ev)
                inst.sync_info = mybir.SyncInfo(
                    on_wait=[si.on_wait[-1]], on_update=si.on_update
                )
                changed = True
            out_list.append(inst)
        if changed:
            del blk.instructions[:]
            blk.instructions.extend(out_list)
    return nc


def build_program():
    nc = bass.Bass()

    # Per-core inputs.
    fb = nc.dram_tensor("fb", [N, D], F32, kind="ExternalInput")
    fw = nc.dram_tensor("fw", [L, D], F32, kind="ExternalInput")
    fm = nc.dram_tensor("fm", [N, N, D], F32, kind="ExternalInput")  # [m, n, d]
    fsb = nc.dram_tensor("fsb", [N, D], F32, kind="ExternalInput")   # f_s bcast
    fsi = nc.dram_tensor("fsi", [N, D], F32, kind="ExternalInput")   # 1/f_s bcast
    # Replicated weights. wqT/bqv are pre-scaled by SCALE on the host.
    wqT = nc.dram_tensor("wqT", [D, D], F32, kind="ExternalInput")   # SCALE*Wq.T
    wkT = nc.dram_tensor("wkT", [D, D], F32, kind="ExternalInput")   # Wk.T
    bqv = nc.dram_tensor("bqv", [D, 1], F32, kind="ExternalInput")   # SCALE*bq
    bkv = nc.dram_tensor("bkv", [D, 1], F32, kind="ExternalInput")
    ident = nc.dram_tensor("ident", [128, 128], F32, kind="ExternalInput")

    out = nc.dram_tensor("out", [N, D], F32, kind="ExternalOutput")

    with tile.TileContext(nc) as tc:
        _emit(nc, tc, fb, fw, fm, fsb, fsi, wqT, wkT, bqv, bkv, ident, out)
    return _legalize_waits(nc)


def _emit(nc, tc, fb, fw, fm, fsb, fsi, wqT, wkT, bqv, bkv, ident, out):
    from contextlib import ExitStack

    ctx = ExitStack()
    with ctx:
        consts = ctx.enter_context(tc.tile_pool(name="consts", bufs=1))
        work = ctx.enter_context(tc.tile_pool(name="work", bufs=2))
        fmpool = ctx.enter_context(tc.tile_pool(name="fmblk", bufs=3))
        pp = ctx.enter_context(tc.tile_pool(name="ppsum", bufs=2, space="PSUM"))
        pacc = ctx.enter_context(tc.tile_pool(name="pacc", bufs=1, space="PSUM"))

        # ---- constant loads -------------------------------------------------
        s_fb = consts.tile([N, D], F32, tag="fb")
        nc.sync.dma_start(out=s_fb, in_=fb[:, :])
        s_fw = consts.tile([L, D], F32, tag="fw")
        nc.sync.dma_start(out=s_fw, in_=fw[:, :])
        s_fsb = consts.tile([N, D], F32, tag="fsb")
        nc.sync.dma_start(out=s_fsb, in_=fsb[:, :])
        s_fsi = consts.tile([N, D], F32, tag="fsi")
        nc.sync.dma_start(out=s_fsi, in_=fsi[:, :])
        s_id = consts.tile([128, 128], F32, tag="ident")
        nc.sync.dma_start(out=s_id, in_=ident[:, :])

        s_wq = []
        s_wk = []
        s_bq = []
        s_bk = []
        for c in range(2):
            t = consts.tile([128, D], F32, tag=f"wq{c}")
            nc.sync.dma_start(out=t, in_=wqT[c * 128:(c + 1) * 128, :])
            s_wq.append(t)
            t = consts.tile([128, D], F32, tag=f"wk{c}")
            nc.sync.dma_start(out=t, in_=wkT[c * 128:(c + 1) * 128, :])
            s_wk.append(t)
            t = consts.tile([128, 1], F32, tag=f"bq{c}")
            nc.gpsimd.dma_start(out=t, in_=bqv[c * 128:(c + 1) * 128, :])
            s_bq.append(t)
            t = consts.tile([128, 1], F32, tag=f"bk{c}")
            nc.gpsimd.dma_start(out=t, in_=bkv[c * 128:(c + 1) * 128, :])
            s_bk.append(t)

        # f_s replicated NB times along free dim: [128, NB, D], built from the
        # [N, D] DRAM copy with a stride-0 middle dim (SWDGE broadcast).
        s_fsrep = consts.tile([N, NB, D], F32, tag="fsrep")
        fsb_ap = fsb[:, :]
        fsb_bcast = bass.AP(
            tensor=fsb_ap.tensor,
            offset=fsb_ap.offset,
            ap=[fsb_ap.ap[0], [0, NB], fsb_ap.ap[1]],
        )
        nc.gpsimd.dma_start(out=s_fsrep, in_=fsb_bcast)

        # ---- attention of f_b over f_w -------------------------------------
        # fbT chunks: [d_chunk=128, n=128]
        s_fbT = []
        for c in range(2):
            pt = pp.tile([128, 128], F32, tag="ptrans")
            nc.tensor.transpose(out=pt, in_=s_fb[:, c * 128:(c + 1) * 128],
                                identity=s_id)
            st = work.tile([128, 128], F32, tag=f"fbT{c}")
            nc.vector.tensor_copy(out=st, in_=pt)
            s_fbT.append(st)

        # qT chunks [d'=128, n=128]; q is pre-scaled by SCALE via wqT/bqv.
        s_qT = []
        for mc in range(2):
            pq = pp.tile([128, 128], F32, tag="pmm")
            for kc in range(2):
                nc.tensor.matmul(
                    out=pq,
                    lhsT=s_wq[kc][:, mc * 128:(mc + 1) * 128],
                    rhs=s_fbT[kc],
                    start=(kc == 0),
                    stop=(kc == 1),
                )
            st = work.tile([128, 128], F32, tag=f"qT{mc}")
            nc.scalar.activation(out=st, in_=pq, func=AF.Identity,
                                 bias=s_bq[mc], scale=1.0)
            s_qT.append(st)

        # fwT chunks: [d_chunk=128, l=30]
        s_fwT = []
        for c in range(2):
            pt = pp.tile([128, L], F32, tag="ptrans")
            nc.tensor.transpose(out=pt, in_=s_fw[:, c * 128:(c + 1) * 128],
                                identity=s_id[:L, :L])
            st = work.tile([128, L], F32, tag=f"fwT{c}")
            nc.vector.tensor_copy(out=st, in_=pt)
            s_fwT.append(st)

        # kT chunks [d'=128, l=30]
        s_kT = []
        for mc in range(2):
            pk = pp.tile([128, L], F32, tag="pmm")
            for kc in range(2):
                nc.tensor.matmul(
                    out=pk,
                    lhsT=s_wk[kc][:, mc * 128:(mc + 1) * 128],
                    rhs=s_fwT[kc],
                    start=(kc == 0),
                    stop=(kc == 1),
                )
            st = work.tile([128, L], F32, tag=f"kT{mc}")
            nc.scalar.activation(out=st, in_=pk, func=AF.Identity,
                                 bias=s_bk[mc], scale=1.0)
            s_kT.append(st)

        # aw logits [n=128, l=30] (already scaled by SCALE)
        p_aw = pp.tile([N, L], F32, tag="pmm")
        for kc in range(2):
            nc.tensor.matmul(out=p_aw, lhsT=s_qT[kc], rhs=s_kT[kc],
                             start=(kc == 0), stop=(kc == 1))

        # softmax over l
        mx1 = work.tile([N, 1], F32, tag="mx1")
        nc.vector.reduce_max(out=mx1, in_=p_aw, axis=AX.X)
        nmx1 = work.tile([N, 1], F32, tag="nmx1")
        nc.scalar.mul(nmx1, mx1, -1.0)
        e_aw = work.tile([N, L], F32, tag="eaw")
        nc.scalar.activation(out=e_aw, in_=p_aw, func=AF.Exp,
                             bias=nmx1, scale=1.0)
        sm1 = work.tile([N, 1], F32, tag="sm1")
        nc.vector.reduce_sum(out=sm1, in_=e_aw, axis=AX.X)
        r1 = work.tile([N, 1], F32, tag="r1")
        nc.vector.reciprocal(out=r1, in_=sm1)
        naw = work.tile([N, L], F32, tag="naw")
        nc.vector.tensor_scalar_mul(naw, e_aw, r1)

        # f_baq = naw @ f_w : transpose naw -> [l, n], then PE
        p_awT = pp.tile([L, N], F32, tag="ptrans")
        nc.tensor.transpose(out=p_awT, in_=naw, identity=s_id)
        s_awT = work.tile([L, N], F32, tag="awT")
        nc.vector.tensor_copy(out=s_awT, in_=p_awT)

        p_fbaq = pp.tile([N, D], F32, tag="pmm")
        nc.tensor.matmul(out=p_fbaq, lhsT=s_awT, rhs=s_fw,
                         start=True, stop=True)

        # f_bq = f_b * (f_baq + f_s)
        s_t = work.tile([N, D], F32, tag="t")
        nc.vector.tensor_add(s_t, p_fbaq, s_fsb)
        s_fbq = work.tile([N, D], F32, tag="fbq")
        nc.vector.tensor_mul(s_fbq, s_t, s_fb)

        # A = softmax(SCALE * f_bq f_bq^T) over m
        s_fbqT = []
        for c in range(2):
            pt = pp.tile([128, 128], F32, tag="ptrans")
            nc.tensor.transpose(out=pt, in_=s_fbq[:, c * 128:(c + 1) * 128],
                                identity=s_id)
            st = work.tile([128, 128], F32, tag=f"fbqT{c}")
            nc.vector.tensor_copy(out=st, in_=pt)
            s_fbqT.append(st)

        p_A = pp.tile([N, N], F32, tag="pmm")
        for kc in range(2):
            nc.tensor.matmul(out=p_A, lhsT=s_fbqT[kc], rhs=s_fbqT[kc],
                             start=(kc == 0), stop=(kc == 1))

        mx2 = work.tile([N, 1], F32, tag="mx2")
        nc.vector.reduce_max(out=mx2, in_=p_A, axis=AX.X)
        nmx2 = work.tile([N, 1], F32, tag="nmx2")
        nc.scalar.mul(nmx2, mx2, -SCALE)
        e_A = work.tile([N, N], F32, tag="eA")
        nc.scalar.activation(out=e_A, in_=p_A, func=AF.Exp,
                             bias=nmx2, scale=SCALE)
        sm2 = work.tile([N, 1], F32, tag="sm2")
        nc.vector.reduce_sum(out=sm2, in_=e_A, axis=AX.X)
        r2 = work.tile([N, 1], F32, tag="r2")
        nc.vector.reciprocal(out=r2, in_=sm2)
        s_A = work.tile([N, N], F32, tag="A")
        nc.vector.tensor_scalar_mul(s_A, e_A, r2)

        # A^T for the matvec stationaries and f_bb
        p_AT = pp.tile([N, N], F32, tag="ptrans")
        nc.tensor.transpose(out=p_AT, in_=s_A, identity=s_id)
        s_AT = work.tile([N, N], F32, tag="AT")
        nc.vector.tensor_copy(out=s_AT, in_=p_AT)

        # f_bb = A @ f_b
        p_fbb = pacc.tile([N, D], F32, tag="fbb")
        nc.tensor.matmul(out=p_fbb, lhsT=s_AT, rhs=s_fb, start=True, stop=True)

        # ---- streamed gated aggregation over f_m ---------------------------
        # Per-row matvecs f_bm[n,:] = A[n,:] @ H_n need the A column as the
        # (cheap) stationary and the 16KB H slab as the moving operand, but a
        # PE matmul output must start at partition 0/32/64.  So expand A^T
        # into AZ[m, n*32 + c] = A^T[m, n] * (c == n % 32): the stationary for
        # row n is the 32-column slab AZ[:, n*32:(n+1)*32] whose single
        # nonzero column places the result at PSUM partition n % 32, and 32
        # consecutive rows accumulate into one [32, D] PSUM tile.
        GRP = 32
        s_AZ = consts.tile([128, N * GRP], F32, tag="AZ")
        nc.gpsimd.memset(s_AZ, 0.0)
        az_ap = s_AZ[:, :]
        az_diag = bass.AP(
            tensor=az_ap.tensor,
            offset=az_ap.offset,
            ap=[az_ap.ap[0], [GRP * GRP, N // GRP], [GRP + 1, GRP]],
        )
        at_ap = s_AT[:, :]
        at_grp = bass.AP(
            tensor=at_ap.tensor,
            offset=at_ap.offset,
            ap=[at_ap.ap[0], [GRP, N // GRP], [1, GRP]],
        )
        nc.vector.tensor_copy(out=az_diag, in_=at_grp)

        s_fbm = work.tile([N, D], F32, tag="fbm_s")
        pg = None
        for j in range(NBLK):
            xt = fmpool.tile([128, NB, D], F32, tag="xt")
            nc.sync.dma_start(out=xt, in_=fm[:, j * NB:(j + 1) * NB, :])
            x2 = xt.rearrange("m n d -> m (n d)")
            nc.vector.tensor_mul(x2, x2, s_fsrep.rearrange("m n d -> m (n d)"))
            nc.scalar.activation(out=x2, in_=x2, func=AF.Silu)
            for i in range(NB):
                n = j * NB + i
                g, c = divmod(n, GRP)
                if c == 0:
                    pg = pacc.tile([GRP, D], F32, tag="pg", bufs=2)
                nc.tensor.matmul(
                    out=pg,
                    lhsT=s_AZ[:, n * GRP:(n + 1) * GRP],
                    rhs=xt[:, i, :],
                    start=(c == 0),
                    stop=(c == GRP - 1),
                )
                if c == GRP - 1:
                    nc.vector.tensor_copy(
                        out=s_fbm[g * GRP:(g + 1) * GRP, :], in_=pg
                    )

        # ---- combine: out = f_bb + f_b + f_bm / f_s ------------------------
        s_o1 = work.tile([N, D], F32, tag="o1")
        nc.vector.tensor_mul(s_o1, s_fbm, s_fsi)
        s_o2 = work.tile([N, D], F32, tag="o2")
        nc.vector.tensor_add(s_o2, p_fbb, s_fb)
        s_out = work.tile([N, D], F32, tag="oo")
        nc.vector.tensor_add(s_out, s_o1, s_o2)
        nc.sync.dma_start(out=out[:, :], in_=s_out)


def get_program():
    global _CACHED_NC
    if _CACHED_NC is None:
        _CACHED_NC = build_program()
    return _CACHED_NC


def make_in_maps(inputs):
    f_b = np.ascontiguousarray(np.asarray(inputs["f_b"], np.float32))
    f_w = np.ascontiguousarray(np.asarray(inputs["f_w"], np.float32))
    f_s = np.ascontiguousarray(np.asarray(inputs["f_s"], np.float32))
    f_m = np.asarray(inputs["f_m"], np.float32)
    Wq = np.asarray(inputs["Wq"], np.float32)
    bq = np.asarray(inputs["bq"], np.float32)
    Wk = np.asarray(inputs["Wk"], np.float32)
    bk = np.asarray(inputs["bk"], np.float32)

    wqT = np.ascontiguousarray(Wq.T * SCALE)
    wkT = np.ascontiguousarray(Wk.T)
    bqv = np.ascontiguousarray((bq * SCALE).reshape(D, 1))
    bkv = np.ascontiguousarray(bk.reshape(D, 1))
    ident = np.eye(128, dtype=np.float32)

    in_maps = []
    for b in range(B):
        in_maps.append({
            "fb": f_b[b],
            "fw": f_w[b],
            # [n, m, d] -> [m, n, d] so block DMAs are contiguous 16KB runs
            "fm": np.ascontiguousarray(f_m[b].transpose(1, 0, 2)),
            "fsb": np.ascontiguousarray(np.broadcast_to(f_s[b], (N, D))),
            "fsi": np.ascontiguousarray(np.broadcast_to(1.0 / f_s[b], (N, D))),
            "wqT": wqT,
            "wkT": wkT,
            "bqv": bqv,
            "bkv": bkv,
            "ident": ident,
        })
    return in_maps


def kernel(**inputs) -> np.ndarray:
    nc = get_program()
    in_maps = make_in_maps(inputs)
    res = bass_utils.run_bass_kernel_spmd(nc, in_maps, list(range(B))).results
    return np.stack([np.asarray(res[b]["out"], np.float32) for b in range(B)],
                    axis=0)


# revision 13
# speedup vs baseline: 1.5177x; 1.1242x over previous
"""Trainium2 Bass kernel for nn_BoundaryUnit (gnn_message_passing).

Computation (per batch b):
    q  = f_b @ Wq.T + bq                  [N,D]
    k  = f_w @ Wk.T + bk                  [L,D]
    aw = softmax(scale * q k^T)           [N,L]   (query_mask == ones)
    f_baq = aw @ f_w                      [N,D]
    f_bq  = f_b * (f_baq + f_s)           [N,D]
    A  = softmax(scale * f_bq f_bq^T)     [N,N]   (length_mask == ones)
    f_bb = A @ f_b                        [N,D]
    f_bm = einsum('nm,nmd->nd', A, f_m * sigmoid(f_m * f_s))
    out  = f_bb + f_b + f_bm

Sharding: data-parallel over batch B=8 across the 8 NeuronCores; each core
computes one batch independently (no collectives).

The dominant cost is streaming f_m (16.75 MB/core).  Host pre-transposes
f_m to [m, n, d] so each 2 MB block DMA is made of contiguous 16 KB
per-partition runs (full HBM bandwidth).  The gated product
f_m * sigmoid(f_m * f_s) is computed as silu(f_s * f_m) / f_s — one DVE
multiply + one ACT pass over the stream — with the 1/f_s correction
applied to the small [N,D] accumulator at the end.  The A-weighted
reduction over m runs on the PE as per-row matvecs (stationary = column
of A^T, moving = the silu'd [m, d] slab), accumulating rows directly in
PSUM.
"""

import math
import sys

import numpy as np

sys.path.insert(0, "/opt/trn_rl_repo")

import concourse.bass as bass  # noqa: E402
import concourse.tile as tile  # noqa: E402
from concourse import bass_utils, mybir  # noqa: E402

B, N, L, D = 8, 128, 30, 256
NB = 16            # n-rows of f_m handled per streamed block
NBLK = N // NB     # 8 blocks
SCALE = 1.0 / math.sqrt(D)
F32 = mybir.dt.float32
AF = mybir.ActivationFunctionType
AX = mybir.AxisListType

_CACHED_NC = None


def _legalize_waits(nc):
    """Split multi-wait instructions: this walrus build accepts at most ONE
    sync-wait per data instruction, so move extra waits onto standalone
    InstEventSemaphore (the same lowering wait_ge uses) just before it."""
    for blk in nc.main_func.blocks:
        insts = list(blk.instructions)
        out_list = []
        changed = False
        for inst in insts:
            si = inst.sync_info
            if si is not None and len(si.on_wait) > 1:
                for w in si.on_wait[:-1]:
                    ev = mybir.InstEventSemaphore(
                        name=nc.get_next_instruction_name(), ins=[], outs=[]
                    )
                    ev.engine = inst.engine
                    ev.sync_info = mybir.SyncInfo(on_wait=[w], on_update=[])
                    nc.register_instruction(ev)
                    out_list.append(ev)
                inst.sync_info = mybir.SyncInfo(
                    on_wait=[si.on_wait[-1]], on_update=si.on_update
                )
                changed = True
            out_list.append(inst)
        if changed:
            del blk.instructions[:]
            blk.instructions.extend(out_list)
    return nc


def build_program():
    nc = bass.Bass()

    # Per-core inputs.
    fb = nc.dram_tensor("fb", [N, D], F32, kind="ExternalInput")
    fw = nc.dram_tensor("fw", [L, D], F32, kind="ExternalInput")
    fm = nc.dram_tensor("fm", [N, N, D], F32, kind="ExternalInput")  # [m, n, d]
    fsb = nc.dram_tensor("fsb", [N, D], F32, kind="ExternalInput")   # f_s bcast
    fsi = nc.dram_tensor("fsi", [N, D], F32, kind="ExternalInput")   # 1/f_s bcast
    # Replicated weights. wqT/bqv are pre-scaled by SCALE on the host.
    wqT = nc.dram_tensor("wqT", [D, D], F32, kind="ExternalInput")   # SCALE*Wq.T
    wkT = nc.dram_tensor("wkT", [D, D], F32, kind="ExternalInput")   # Wk.T
    bqv = nc.dram_tensor("bqv", [D, 1], F32, kind="ExternalInput")   # SCALE*bq
    bkv = nc.dram_tensor("bkv", [D, 1], F32, kind="ExternalInput")
    ident = nc.dram_tensor("ident", [128, 128], F32, kind="ExternalInput")

    out = nc.dram_tensor("out", [N, D], F32, kind="ExternalOutput")

    with tile.TileContext(nc) as tc:
        _emit(nc, tc, fb, fw, fm, fsb, fsi, wqT, wkT, bqv, bkv, ident, out)
    return _legalize_waits(nc)


def _emit(nc, tc, fb, fw, fm, fsb, fsi, wqT, wkT, bqv, bkv, ident, out):
    from contextlib import ExitStack

    ctx = ExitStack()
    with ctx:
        consts = ctx.enter_context(tc.tile_pool(name="consts", bufs=1))
        work = ctx.enter_context(tc.tile_pool(name="work", bufs=2))
        fmpool = ctx.enter_context(tc.tile_pool(name="fmblk", bufs=4))
        pp = ctx.enter_context(tc.tile_pool(name="ppsum", bufs=2, space="PSUM"))
        pacc = ctx.enter_context(tc.tile_pool(name="pacc", bufs=1, space="PSUM"))

        # AZ (block-expanded A^T, see below) - zero it early on DVE so the
        # diagonal scatter after the attention chain is all that remains.
        GRP = 32
        s_AZ = consts.tile([128, N * GRP], F32, tag="AZ")
        nc.vector.memset(s_AZ, 0.0)

        # ---- constant loads -------------------------------------------------
        s_fb = consts.tile([N, D], F32, tag="fb")
        nc.sync.dma_start(out=s_fb, in_=fb[:, :])
        s_fw = consts.tile([L, D], F32, tag="fw")
        nc.sync.dma_start(out=s_fw, in_=fw[:, :])
        s_fsb = consts.tile([N, D], F32, tag="fsb")
        nc.sync.dma_start(out=s_fsb, in_=fsb[:, :])
        s_fsi = consts.tile([N, D], F32, tag="fsi")
        nc.sync.dma_start(out=s_fsi, in_=fsi[:, :])
        s_id = consts.tile([128, 128], F32, tag="ident")
        nc.sync.dma_start(out=s_id, in_=ident[:, :])

        s_wq = []
        s_wk = []
        s_bq = []
        s_bk = []
        for c in range(2):
            t = consts.tile([128, D], F32, tag=f"wq{c}")
            nc.sync.dma_start(out=t, in_=wqT[c * 128:(c + 1) * 128, :])
            s_wq.append(t)
            t = consts.tile([128, D], F32, tag=f"wk{c}")
            nc.sync.dma_start(out=t, in_=wkT[c * 128:(c + 1) * 128, :])
            s_wk.append(t)
            t = consts.tile([128, 1], F32, tag=f"bq{c}")
            nc.gpsimd.dma_start(out=t, in_=bqv[c * 128:(c + 1) * 128, :])
            s_bq.append(t)
            t = consts.tile([128, 1], F32, tag=f"bk{c}")
            nc.gpsimd.dma_start(out=t, in_=bkv[c * 128:(c + 1) * 128, :])
            s_bk.append(t)

        # f_s replicated NB times along free dim: [128, NB, D], built from the
        # [N, D] DRAM copy with a stride-0 middle dim (SWDGE broadcast).
        s_fsrep = consts.tile([N, NB, D], F32, tag="fsrep")
        fsb_ap = fsb[:, :]
        fsb_bcast = bass.AP(
            tensor=fsb_ap.tensor,
            offset=fsb_ap.offset,
            ap=[fsb_ap.ap[0], [0, NB], fsb_ap.ap[1]],
        )
        nc.gpsimd.dma_start(out=s_fsrep, in_=fsb_bcast)

        # ---- attention of f_b over f_w -------------------------------------
        # fbT chunks: [d_chunk=128, n=128]
        s_fbT = []
        for c in range(2):
            pt = pp.tile([128, 128], F32, tag="ptrans")
            nc.tensor.transpose(out=pt, in_=s_fb[:, c * 128:(c + 1) * 128],
                                identity=s_id)
            st = work.tile([128, 128], F32, tag=f"fbT{c}")
            nc.vector.tensor_copy(out=st, in_=pt)
            s_fbT.append(st)

        # qT chunks [d'=128, n=128]; q is pre-scaled by SCALE via wqT/bqv.
        s_qT = []
        for mc in range(2):
            pq = pp.tile([128, 128], F32, tag="pmm")
            for kc in range(2):
                nc.tensor.matmul(
                    out=pq,
                    lhsT=s_wq[kc][:, mc * 128:(mc + 1) * 128],
                    rhs=s_fbT[kc],
                    start=(kc == 0),
                    stop=(kc == 1),
                )
            st = work.tile([128, 128], F32, tag=f"qT{mc}")
            nc.vector.tensor_scalar_add(st, pq, s_bq[mc])
            s_qT.append(st)

        # fwT chunks: [d_chunk=128, l=30]
        s_fwT = []
        for c in range(2):
            pt = pp.tile([128, L], F32, tag="ptrans")
            nc.tensor.transpose(out=pt, in_=s_fw[:, c * 128:(c + 1) * 128],
                                identity=s_id[:L, :L])
            st = work.tile([128, L], F32, tag=f"fwT{c}")
            nc.vector.tensor_copy(out=st, in_=pt)
            s_fwT.append(st)

        # kT chunks [d'=128, l=30]
        s_kT = []
        for mc in range(2):
            pk = pp.tile([128, L], F32, tag="pmm")
            for kc in range(2):
                nc.tensor.matmul(
                    out=pk,
                    lhsT=s_wk[kc][:, mc * 128:(mc + 1) * 128],
                    rhs=s_fwT[kc],
                    start=(kc == 0),
                    stop=(kc == 1),
                )
            st = work.tile([128, L], F32, tag=f"kT{mc}")
            nc.vector.tensor_scalar_add(st, pk, s_bk[mc])
            s_kT.append(st)

        # aw logits [n=128, l=30] (already scaled by SCALE)
        p_aw = pp.tile([N, L], F32, tag="pmm")
        for kc in range(2):
            nc.tensor.matmul(out=p_aw, lhsT=s_qT[kc], rhs=s_kT[kc],
                             start=(kc == 0), stop=(kc == 1))

        # softmax over l
        mx1 = work.tile([N, 1], F32, tag="mx1")
        nc.vector.reduce_max(out=mx1, in_=p_aw, axis=AX.X)
        nmx1 = work.tile([N, 1], F32, tag="nmx1")
        nc.vector.tensor_scalar_mul(nmx1, mx1, -1.0)
        e_aw = work.tile([N, L], F32, tag="eaw")
        nc.scalar.activation(out=e_aw, in_=p_aw, func=AF.Exp,
                             bias=nmx1, scale=1.0)
        sm1 = work.tile([N, 1], F32, tag="sm1")
        nc.vector.reduce_sum(out=sm1, in_=e_aw, axis=AX.X)
        r1 = work.tile([N, 1], F32, tag="r1")
        nc.vector.reciprocal(out=r1, in_=sm1)
        naw = work.tile([N, L], F32, tag="naw")
        nc.vector.tensor_scalar_mul(naw, e_aw, r1)

        # f_baq = naw @ f_w : transpose naw -> [l, n], then PE
        p_awT = pp.tile([L, N], F32, tag="ptrans")
        nc.tensor.transpose(out=p_awT, in_=naw, identity=s_id)
        s_awT = work.tile([L, N], F32, tag="awT")
        nc.vector.tensor_copy(out=s_awT, in_=p_awT)

        p_fbaq = pp.tile([N, D], F32, tag="pmm")
        nc.tensor.matmul(out=p_fbaq, lhsT=s_awT, rhs=s_fw,
                         start=True, stop=True)

        # f_bq = f_b * (f_baq + f_s)
        s_t = work.tile([N, D], F32, tag="t")
        nc.vector.tensor_add(s_t, p_fbaq, s_fsb)
        s_fbq = work.tile([N, D], F32, tag="fbq")
        nc.vector.tensor_mul(s_fbq, s_t, s_fb)

        # A = softmax(SCALE * f_bq f_bq^T) over m
        s_fbqT = []
        for c in range(2):
            pt = pp.tile([128, 128], F32, tag="ptrans")
            nc.tensor.transpose(out=pt, in_=s_fbq[:, c * 128:(c + 1) * 128],
                                identity=s_id)
            st = work.tile([128, 128], F32, tag=f"fbqT{c}")
            nc.vector.tensor_copy(out=st, in_=pt)
            s_fbqT.append(st)

        p_A = pp.tile([N, N], F32, tag="pmm")
        for kc in range(2):
            nc.tensor.matmul(out=p_A, lhsT=s_fbqT[kc], rhs=s_fbqT[kc],
                             start=(kc == 0), stop=(kc == 1))

        mx2 = work.tile([N, 1], F32, tag="mx2")
        nc.vector.reduce_max(out=mx2, in_=p_A, axis=AX.X)
        nmx2 = work.tile([N, 1], F32, tag="nmx2")
        nc.vector.tensor_scalar_mul(nmx2, mx2, -SCALE)
        e_A = work.tile([N, N], F32, tag="eA")
        i_expA = nc.scalar.activation(out=e_A, in_=p_A, func=AF.Exp,
                                      bias=nmx2, scale=SCALE)
        sm2 = work.tile([N, 1], F32, tag="sm2")
        nc.vector.reduce_sum(out=sm2, in_=e_A, axis=AX.X)
        r2 = work.tile([N, 1], F32, tag="r2")
        nc.vector.reciprocal(out=r2, in_=sm2)
        s_A = work.tile([N, N], F32, tag="A")
        nc.vector.tensor_scalar_mul(s_A, e_A, r2)

        # A^T for the matvec stationaries and f_bb
        p_AT = pp.tile([N, N], F32, tag="ptrans")
        nc.tensor.transpose(out=p_AT, in_=s_A, identity=s_id)
        s_AT = work.tile([N, N], F32, tag="AT")
        nc.vector.tensor_copy(out=s_AT, in_=p_AT)

        # f_bb = A @ f_b
        p_fbb = pacc.tile([N, D], F32, tag="fbb")
        nc.tensor.matmul(out=p_fbb, lhsT=s_AT, rhs=s_fb, start=True, stop=True)

        # ---- streamed gated aggregation over f_m ---------------------------
        # Per-row matvecs f_bm[n,:] = A[n,:] @ H_n need the A column as the
        # (cheap) stationary and the 16KB H slab as the moving operand, but a
        # PE matmul output must start at partition 0/32/64.  So expand A^T
        # into AZ[m, n*32 + c] = A^T[m, n] * (c == n % 32): the stationary for
        # row n is the 32-column slab AZ[:, n*32:(n+1)*32] whose single
        # nonzero column places the result at PSUM partition n % 32, and 32
        # consecutive rows accumulate into one [32, D] PSUM tile.
        az_ap = s_AZ[:, :]
        az_diag = bass.AP(
            tensor=az_ap.tensor,
            offset=az_ap.offset,
            ap=[az_ap.ap[0], [GRP * GRP, N // GRP], [GRP + 1, GRP]],
        )
        at_ap = s_AT[:, :]
        at_grp = bass.AP(
            tensor=at_ap.tensor,
            offset=at_ap.offset,
            ap=[at_ap.ap[0], [GRP, N // GRP], [1, GRP]],
        )
        nc.vector.tensor_copy(out=az_diag, in_=at_grp)

        s_fbm = work.tile([N, D], F32, tag="fbm_s")
        pg = None
        for j in range(NBLK):
            xt = fmpool.tile([128, NB, D], F32, tag="xt")
            nc.sync.dma_start(out=xt, in_=fm[:, j * NB:(j + 1) * NB, :])
            x2 = xt.rearrange("m n d -> m (n d)")
            nc.vector.tensor_mul(x2, x2, s_fsrep.rearrange("m n d -> m (n d)"))
            i_silu = nc.scalar.activation(out=x2, in_=x2, func=AF.Silu)
            if j == 0:
                # Keep ACT in [Exp, Exp, Silu...] order: one exp->silu table
                # switch instead of thrashing back and forth.
                tile.add_dep_helper(i_silu.ins, i_expA.ins, False,
                                    "act table-set ordering")
            for i in range(NB):
                n = j * NB + i
                g, c = divmod(n, GRP)
                if c == 0:
                    pg = pacc.tile([GRP, D], F32, tag="pg", bufs=2)
                nc.tensor.matmul(
                    out=pg,
                    lhsT=s_AZ[:, n * GRP:(n + 1) * GRP],
                    rhs=xt[:, i, :],
                    start=(c == 0),
                    stop=(c == GRP - 1),
                )
                if c == GRP - 1:
                    nc.vector.tensor_copy(
                        out=s_fbm[g * GRP:(g + 1) * GRP, :], in_=pg
                    )

        # ---- combine: out = f_bb + f_b + f_bm / f_s ------------------------
        s_o1 = work.tile([N, D], F32, tag="o1")
        nc.vector.tensor_mul(s_o1, s_fbm, s_fsi)
        s_o2 = work.tile([N, D], F32, tag="o2")
        nc.vector.tensor_add(s_o2, p_fbb, s_fb)
        s_out = work.tile([N, D], F32, tag="oo")
        nc.vector.tensor_add(s_out, s_o1, s_o2)
        nc.sync.dma_start(out=out[:, :], in_=s_out)


def get_program():
    global _CACHED_NC
    if _CACHED_NC is None:
        _CACHED_NC = build_program()
    return _CACHED_NC


def make_in_maps(inputs):
    f_b = np.ascontiguousarray(np.asarray(inputs["f_b"], np.float32))
    f_w = np.ascontiguousarray(np.asarray(inputs["f_w"], np.float32))
    f_s = np.ascontiguousarray(np.asarray(inputs["f_s"], np.float32))
    f_m = np.asarray(inputs["f_m"], np.float32)
    Wq = np.asarray(inputs["Wq"], np.float32)
    bq = np.asarray(inputs["bq"], np.float32)
    Wk = np.asarray(inputs["Wk"], np.float32)
    bk = np.asarray(inputs["bk"], np.float32)

    wqT = np.ascontiguousarray(Wq.T * SCALE)
    wkT = np.ascontiguousarray(Wk.T)
    bqv = np.ascontiguousarray((bq * SCALE).reshape(D, 1))
    bkv = np.ascontiguousarray(bk.reshape(D, 1))
    ident = np.eye(128, dtype=np.float32)

    in_maps = []
    for b in range(B):
        in_maps.append({
            "fb": f_b[b],
            "fw": f_w[b],
            # [n, m, d] -> [m, n, d] so block DMAs are contiguous 16KB runs
            "fm": np.ascontiguousarray(f_m[b].transpose(1, 0, 2)),
            "fsb": np.ascontiguousarray(np.broadcast_to(f_s[b], (N, D))),
            "fsi": np.ascontiguousarray(np.broadcast_to(1.0 / f_s[b], (N, D))),
            "wqT": wqT,
            "wkT": wkT,
            "bqv": bqv,
            "bkv": bkv,
            "ident": ident,
        })
    return in_maps


def kernel(**inputs) -> np.ndarray:
    nc = get_program()
    in_maps = make_in_maps(inputs)
    res = bass_utils.run_bass_kernel_spmd(nc, in_maps, list(range(B))).results
    return np.stack([np.asarray(res[b]["out"], np.float32) for b in range(B)],
                    axis=0)
